# revision 55
# baseline (speedup 1.0000x reference)
"""Trainium2 Bass kernel for ActionExpertCrossBlock (dense transformer block
with GQA cross-attention + SwiGLU FFN), data-parallel over batch on 8 cores.

Contract: kernel(**inputs) takes the FULL fp32 inputs as produced by
setup_inputs() and returns the FULL [8, 512, 1024] fp32 output.

Per-core computation (batch element b):
  h   = rmsnorm(x) * ln1_w
  q   = rope((h @ Wq.T).reshape(L, 8, 256))
  k   = text_k @ Wk.T          (single KV head, shared by all 8 Q heads)
  v   = text_v @ Wv.T
  s_h = q_h @ k.T / 16         -> softmax over context
  ctx = attn @ v ; x2 = ctx @ Wo.T + x
  h2  = rmsnorm(x2) * ln2_w
  out = (silu(h2@Wg.T) * (h2@Wu.T)) @ Wd.T + x2

Precision: the whole attention path (q/k/v projections, scores, exp,
ctx, o-proj) runs in fp8-e4m3 with DoubleRow double-pumped matmuls
(2x PE throughput, K=256 per pass).  The unnormalized softmax runs as
exp(s/16 - 4) so the fp8 range [2^-10, 240] is never exceeded (score
max is ~8 over the full batch); the constant shift cancels in the
softmax ratio because the denominator sums the SAME fp8 exp values.
The FFN (gate/up/down) stays bf16: its outputs are O(1) against a
residual stream whose global absmax is ~6.4, so fp8 there would blow
the 2e-2 absmax gate (measured 0.02-0.04 per matmul in simulation),
while the attention output is tiny relative to the residual and
tolerates fp8 (simulated total rel-err 0.013 vs 0.0025 all-bf16).
x is carried in bf16 (residual quantization ~2e-3 of the gate scale).

Schedule: the attention phase is jointly ACT/PE-bound (after the exp
pair-batching below), the FFN phase PE-bound.  All moving dims stay at
the full 512 columns -- splitting them raises the fixed per-instruction
overheads (ACT ~250ns setup per op, LDWEIGHTS reloads) by more than any
attention/FFN overlap it buys (measured +50us for an L-split variant).
Both score blocks of a context pair accumulate into one 2-bank PSUM
tile so a single 1024-wide ACTIVATE computes their exp (16us less ACT).
Inputs stream in the order wk/wv/tk/x/tv/wq (each dma_start costs
~1.1us of Sync-queue issue time, so the startup loads are chunked
coarsely); q-projection + RoPE for head h+2 is software-pipelined INTO
head h's attention iteration (PSUM slotC + a fast DVE copy to SBUF so
RoPE's DVE work never holds a PSUM bank); the o-proj/gate/up weights
prefetch during the DMA-idle attention phase; v-proj PSUM copies ride
on DVE behind RoPE h0/h1 to keep ACT clear for the exp stream.  Scores
are computed TRANSPOSED ([c, l]) so attn@v needs no transpose of the
attention matrix; the softmax partition-sum is an all-ones fp8
DoubleRow matmul which also broadcasts the denominator to every
partition.
"""
import sys

sys.path.insert(0, "/opt/trn_rl_repo")

import numpy as np
import ml_dtypes

import concourse.bass as bass
from concourse import bacc
import concourse.mybir as mybir
import concourse.tile as tile
from concourse.masks import make_identity
from concourse.bass_utils import run_bass_kernel_spmd

import os as _os

P = 128
B, L, D = 8, 512, 1024
QH, HD = 8, 256
E = 256        # kv dim (1 head x 256)
LC = 2048      # context length
F = 4096       # ffn dim
O = QH * HD    # 2048
LT, DT, OT, CT, FTL = L // P, D // P, O // P, LC // P, F // P  # 4 8 16 16 32
CP = CT // 2   # context pairs for DoubleRow
f32, bf16 = mybir.dt.float32, mybir.dt.bfloat16
f8 = mybir.dt.float8e4
EPS = float(np.finfo(np.float32).eps)
EBIAS = -4.0   # exp(s/16 + EBIAS): keeps fp8 exp in (2^-10, 240)
NPRE = 10      # gate/up weight columns prefetched during attention
EXPF = mybir.ActivationFunctionType.Exp
SQRT = mybir.ActivationFunctionType.Sqrt
COPYF = mybir.ActivationFunctionType.Copy
DR = mybir.MatmulPerfMode.DoubleRow
MUL = mybir.AluOpType.mult
SUB = mybir.AluOpType.subtract
ADD = mybir.AluOpType.add


def _rope_tables():
    # Match reference _rope numerics (fp32 ops) for d=256, l=512.
    d2 = HD // 2
    ts = (10000.0 ** (2.0 / HD * np.arange(d2, dtype=np.float32))).astype(np.float32)
    rad = (np.arange(L, dtype=np.float32)[None, :] / ts[:, None]).astype(np.float32)
    return np.cos(rad).astype(np.float32), np.sin(rad).astype(np.float32)  # [128, 512]


def build_program():
    nc = bacc.Bacc()
    x_d = nc.dram_tensor("x", [L, D], bf16, kind="ExternalInput")
    tkT_d = nc.dram_tensor("tkT", [E, LC], f8, kind="ExternalInput")
    tvT_d = nc.dram_tensor("tvT", [E, LC], f8, kind="ExternalInput")
    wqT_d = nc.dram_tensor("wqT", [D, O], f8, kind="ExternalInput")
    wkT_d = nc.dram_tensor("wkT", [E, E], f8, kind="ExternalInput")
    wvT_d = nc.dram_tensor("wvT", [E, E], f8, kind="ExternalInput")
    woT_d = nc.dram_tensor("woT", [O, D], f8, kind="ExternalInput")
    wgT_d = nc.dram_tensor("wgT", [D, F], bf16, kind="ExternalInput")
    wuT_d = nc.dram_tensor("wuT", [D, F], bf16, kind="ExternalInput")
    wdT_d = nc.dram_tensor("wdT", [F, D], bf16, kind="ExternalInput")
    out_d = nc.dram_tensor("out", [L, D], f32, kind="ExternalOutput")

    ver = np.array([int(_os.environ.get("ANT_KVER", "6"))], np.int32)
    nc.inline_tensor(ver, f"kver_{int(ver[0])}")
    cos_np, sin_np = _rope_tables()
    cosT_d = nc.inline_tensor(cos_np.astype(ml_dtypes.bfloat16), "cosT")
    sinT_d = nc.inline_tensor(sin_np.astype(ml_dtypes.bfloat16), "sinT")

    with tile.TileContext(nc) as tc:
        build_tile_kernel(
            tc, x_d, tkT_d, tvT_d, wqT_d, wkT_d, wvT_d, woT_d, wgT_d, wuT_d,
            wdT_d, cosT_d, sinT_d, out_d,
        )
    nc.compile()
    return nc


def _rmsnorm_lt(nc, pool, x_lt, dst_lt, eps_sb, tag, lt):
    """dst_lt = x_lt * rsqrt(mean(x_lt^2) + eps); sum-of-squares on ACT."""
    sq = pool.tile([P, D], f32, tag=f"{tag}_sq", bufs=2, name=f"{tag}sq{lt}")
    ssum = pool.tile([P, 1], f32, tag=f"{tag}_sum", bufs=2, name=f"{tag}sm{lt}")
    nc.scalar.activation(sq, x_lt, mybir.ActivationFunctionType.Square,
                         accum_out=ssum)
    std = pool.tile([P, 1], f32, tag=f"{tag}_std", bufs=2, name=f"{tag}sd{lt}")
    nc.scalar.activation(std, ssum, SQRT, scale=1.0 / D, bias=eps_sb)
    rstd = pool.tile([P, 1], f32, tag=f"{tag}_rstd", bufs=2, name=f"{tag}rs{lt}")
    nc.vector.reciprocal(rstd, std)
    nc.vector.tensor_scalar_mul(dst_lt, x_lt, rstd)


def build_tile_kernel(tc, x_d, tkT_d, tvT_d, wqT_d, wkT_d, wvT_d, woT_d,
                      wgT_d, wuT_d, wdT_d, cosT_d, sinT_d, out_d):
    nc = tc.nc

    consts = tc.alloc_tile_pool(name="consts", bufs=1)
    persist2 = tc.alloc_tile_pool(name="persist2", bufs=1)  # x2: lives to the end
    persistH = tc.alloc_tile_pool(name="persistH", bufs=1)  # h2T + wg/wu prefetch
    persist1 = tc.alloc_tile_pool(name="persist1", bufs=1)  # dead after o-proj

    ident = consts.tile([P, P], bf16)
    make_identity(nc, ident)
    ones8 = consts.tile([P, 2, P], f8)
    nc.vector.memset(ones8, 1.0)
    eps_sb = consts.tile([P, 1], f32)
    nc.vector.memset(eps_sb, EPS)
    ebias_sb = consts.tile([P, 1], f32)
    nc.vector.memset(ebias_sb, EBIAS)
    cosT = consts.tile([P, L], bf16)
    sinT = consts.tile([P, L], bf16)

    # ---- persistent activations (split into per-slice tiles so consumers
    # depend only on the slices they read, not on whole-tensor last-writes) ----
    qT_h = [persist1.tile([P, 2, L], f8, tag="qT", bufs=QH, name=f"qT{h}")
            for h in range(QH)]
    kT = persist1.tile([P, 2, LC], f8)
    v8 = persist1.tile([P, CP, 2, E], f8)
    ctxT_h = [persist1.tile([P, 2, L], f8, tag="ctxT", bufs=QH, name=f"ctxT{h}")
              for h in range(QH)]
    xall = persist1.tile([P, LT, D], bf16)
    hT = persist1.tile([P, DT, L], f8)
    x2_sb = persist2.tile([P, LT, D], f32)

    pexp = tc.alloc_tile_pool(name="pexp", bufs=1)   # attention exp/recip tiles
    pq = tc.alloc_tile_pool(name="ph_q", bufs=1)     # wq + rope temps
    pa = tc.alloc_tile_pool(name="ph_a", bufs=1)     # startup-only kv inputs
    psum = tc.alloc_tile_pool(name="psum", bufs=2, space="PSUM")

    # ---- input DMAs: each dma_start costs ~1.1us of queue ISSUE time, so
    # the startup-critical loads are spread over three parallel HWDGE rings
    # (Sync: k-side, ACT: x + q weights, DVE: v-side) and chunked coarsely ----
    tk_r = tkT_d.ap().rearrange("(ft p) c -> p ft c", p=P)
    tv_r = tvT_d.ap().rearrange("(ft p) c -> p ft c", p=P)
    x_r = x_d.ap().rearrange("(lt p) d -> p lt d", p=P)
    wq_r = wqT_d.ap().rearrange("(dt p) o -> p dt o", p=P)
    # x first: it feeds the longest pre-attention chain (norm1 -> transpose
    # -> q-proj -> rope) and each issue slot ahead of it costs ~1.1us
    for half in range(2):
        nc.sync.dma_start(xall[:, 2 * half:2 * half + 2, :],
                          x_r[:, 2 * half:2 * half + 2, :])
    nc.sync.dma_start(cosT, cosT_d.ap())
    nc.sync.dma_start(sinT, sinT_d.ap())
    wk_sb = pa.tile([P, 2, E], f8)
    nc.sync.dma_start(wk_sb, wkT_d.ap().rearrange("(ft p) e -> p ft e", p=P))
    wv_sb = pa.tile([P, 2, E], f8)
    nc.sync.dma_start(wv_sb, wvT_d.ap().rearrange("(ft p) e -> p ft e", p=P))
    wq_c = [pq.tile([P, DT, 2 * P], f8, tag="wq", bufs=QH, name=f"wq{h}")
            for h in range(QH)]
    for h in range(2):
        nc.sync.dma_start(wq_c[h], wq_r[:, :, h * 2 * P:(h + 1) * 2 * P])
    tk_c = []
    for cc in range(2):
        t = pa.tile([P, 2, 1024], f8, tag="tk", bufs=2, name=f"tk{cc}")
        nc.sync.dma_start(t, tk_r[:, :, cc * 1024:(cc + 1) * 1024])
        tk_c.append(t)
    tv_c = []
    for cc in range(2):
        t = pa.tile([P, 2, 1024], f8, tag="tv", bufs=2, name=f"tv{cc}")
        nc.sync.dma_start(t, tv_r[:, :, cc * 1024:(cc + 1) * 1024])
        tv_c.append(t)
    for h in range(2, QH):
        nc.sync.dma_start(wq_c[h], wq_r[:, :, h * 2 * P:(h + 1) * 2 * P])

    # ====== rmsnorm1 + transpose first (x-paced) ====
    for lt in range(LT):
        h_lt = pq.tile([P, D], bf16, tag="h_bf", bufs=2, name=f"hbf{lt}")
        _rmsnorm_lt(nc, pq, xall[:, lt, :], h_lt, eps_sb, "n1", lt)
        for dt in range(DT):
            tp = psum.tile([P, P], bf16, tag="slotA", bufs=2, name=f"atp{lt}_{dt}")
            nc.tensor.transpose(tp, h_lt[:, dt * P:(dt + 1) * P], ident)
            nc.vector.tensor_copy(hT[:, dt, lt * P:(lt + 1) * P], tp)

    # ====== k projection (tk-paced, overlaps the norm/transpose chain) ====
    for cc in range(LC // 512):
        for et in range(2):
            psk = psum.tile([P, 512], f32, tag="slotC", bufs=2, name=f"psk{et}_{cc}")
            nc.tensor.matmul(psk, wk_sb[:, :, et * P:(et + 1) * P],
                             tk_c[cc // 2][:, :, (cc % 2) * 512:(cc % 2 + 1) * 512],
                             start=True, stop=True, perf_mode=DR)
            nc.scalar.activation(kT[:, et, cc * 512:(cc + 1) * 512], psk, COPYF)

    # ====== q projection (-> qT, [o, l] layout) + RoPE, per head.
    # psq shares PSUM slotC with the softmax denominator; a fast DVE copy
    # moves each half to SBUF so RoPE's DVE work never holds the PSUM bank. ==
    def emit_qproj(h):
        qsb = []
        for half in range(2):
            psq = psum.tile([P, L], f32, tag="slotC", bufs=2, name=f"psq{2*h+half}")
            for dp in range(DT // 2):
                nc.tensor.matmul(
                    psq, wq_c[h][:, 2 * dp:2 * dp + 2, half * P:(half + 1) * P],
                    hT[:, 2 * dp:2 * dp + 2, :],
                    start=(dp == 0), stop=(dp == DT // 2 - 1), perf_mode=DR)
            qs = pq.tile([P, L], bf16, tag="qsb", bufs=4, name=f"qs{2*h+half}")
            nc.vector.tensor_copy(qs, psq)
            qsb.append(qs)
        # rope: x1 = qsb[0], x2 = qsb[1] ([hd_j, l] layout; tables [j, l]).
        # All-2-byte so DVE runs its 2x mode (bf16 rounding is negligible
        # against the fp8 quantization of qT right after).
        t_a = pq.tile([P, L], bf16, tag="rope_t", bufs=4, name=f"ta{h}")
        nc.vector.tensor_mul(t_a, qsb[0], cosT)
        t_b = pq.tile([P, L], bf16, tag="rope_t", bufs=4, name=f"tb{h}")
        nc.vector.tensor_mul(t_b, qsb[1], sinT)
        nc.vector.tensor_tensor(qT_h[h][:, 0, :], t_a, t_b, SUB)
        t_c = pq.tile([P, L], bf16, tag="rope_t", bufs=4, name=f"tc{h}")
        nc.vector.tensor_mul(t_c, qsb[1], cosT)
        t_d = pq.tile([P, L], bf16, tag="rope_t", bufs=4, name=f"td{h}")
        nc.vector.tensor_mul(t_d, qsb[0], sinT)
        nc.vector.tensor_tensor(qT_h[h][:, 1, :], t_c, t_d, ADD)

    emit_qproj(0)
    emit_qproj(1)

    # ====== v projection (last pre-attention step: v8 is first read by
    # ctx(h0, cp0), a few us into the attention cruise; its PSUM copies ride
    # DVE behind rope h0/h1, keeping ACT clear for the exp stream) ====
    for ct in range(CT):
        psv = psum.tile([P, E], f32, tag="slotC", bufs=2, name=f"psv{ct}")
        nc.tensor.matmul(
            psv, tv_c[ct // 8][:, :, (ct % 8) * P:(ct % 8 + 1) * P], wv_sb,
            start=True, stop=True, perf_mode=DR)
        nc.vector.tensor_copy(v8[:, ct // 2, ct % 2, :], psv)

    # ---- FFN/o-proj weight prefetch: DMA is idle during attention, so gate
    # the bulk loads on DVE-paced dummy writes that fire right about now ----
    pde = tc.alloc_tile_pool(name="ph_de", bufs=1)
    wo8 = pde.tile([P, QH, 2, D], f8)        # 16KB/part
    wgT_r = wgT_d.ap().rearrange("(dt p) f -> p dt f", p=P)
    wuT_r = wuT_d.ap().rearrange("(dt p) f -> p dt f", p=P)
    wg_pre = persistH.tile([P, DT, NPRE * P], bf16)
    wu_pre = persistH.tile([P, DT, NPRE * P], bf16)
    nc.vector.memset(wo8[0:1, 0:1, 0:1, 0:1], 0.0)
    nc.vector.memset(wg_pre[0:1, 0:1, 0:1], 0.0)
    nc.vector.memset(wu_pre[0:1, 0:1, 0:1], 0.0)
    nc.sync.dma_start(wo8, woT_d.ap().rearrange("(h two p) d -> p h two d",
                                                p=P, two=2))
    nc.sync.dma_start(wg_pre, wgT_r[:, :, :NPRE * P])
    nc.sync.dma_start(wu_pre, wuT_r[:, :, :NPRE * P])

    # ---------------- attention (per Q head) ----------------
    for h in range(QH):
        exps = [None] * CP
        psd = psum.tile([P, L], f32, tag="slotC", bufs=2, name=f"psd{h}")
        psc = [
            psum.tile([P, L], f32, tag="slotB", bufs=2, name=f"psc{h}_{et}")
            for et in range(2)
        ]

        def emit_scores(cp, h=h, exps=exps):
            # both score blocks of a context pair land in one 2-bank PSUM
            # tile (separate accumulation groups, each bank-aligned) so a
            # single 1024-wide ACTIVATE computes their exp: ACT's ~250ns
            # per-op setup amortizes over twice the elements
            pss = psum.tile([P, 2, L], f32, tag="slotA", bufs=2,
                            name=f"pss{h}_{cp}")
            for k in range(2):
                nc.tensor.matmul(pss[:, k, :],
                                 kT[:, :, (2 * cp + k) * P:(2 * cp + k + 1) * P],
                                 qT_h[h], start=True, stop=True, perf_mode=DR)
            exps[cp] = pexp.tile([P, 2, L], f8, tag="exp", bufs=4,
                                 name=f"ex{h}_{cp}")
            nc.scalar.activation(exps[cp], pss, EXPF,
                                 scale=1.0 / 16.0, bias=ebias_sb)

        def emit_ctx(cp, psc=psc, exps=exps):
            for et in range(2):
                nc.tensor.matmul(
                    psc[et], v8[:, cp, :, et * P:(et + 1) * P], exps[cp],
                    start=(cp == 0), stop=(cp == CP - 1), perf_mode=DR)

        def emit_den(cp, psd=psd, exps=exps):
            nc.tensor.matmul(psd, ones8, exps[cp],
                             start=(cp == 0), stop=(cp == CP - 1), perf_mode=DR)

        # software pipeline: ctx/den lag scores by one exp-pair so PE never
        # waits on ACT's exp
        for cp in range(CP):
            emit_scores(cp)
            if cp >= 1:
                emit_ctx(cp - 1)
                emit_den(cp - 1)
        emit_ctx(CP - 1)
        emit_den(CP - 1)

        # recip/muls first (they release psd/psc promptly), then head h+2's
        # q-projection rides at end-of-iteration: its RoPE DVE work hides
        # under the next head's ACT-bound exp stream
        recip = pexp.tile([P, L], f32, tag="recip", bufs=2, name=f"rc{h}")
        nc.vector.reciprocal_approx_fast(out=recip, in_=psd)
        for et in range(2):
            nc.vector.tensor_mul(ctxT_h[h][:, et, :], psc[et], recip)

        if h + 2 < QH:
            emit_qproj(h + 2)

    # ------- o-proj + residual, interleaved with norm2/transpose ---------
    h2T = persistH.tile([P, DT, L], bf16)

    def emit_oproj(lt):
        for dc in range(D // 512):
            pso = psum.tile([P, 512], f32, tag="slotC", bufs=2, name=f"pso{lt}_{dc}")
            for h in range(QH):
                nc.tensor.matmul(
                    pso, ctxT_h[h][:, :, lt * P:(lt + 1) * P],
                    wo8[:, h, :, dc * 512:(dc + 1) * 512],
                    start=(h == 0), stop=(h == QH - 1), perf_mode=DR)
            nc.vector.tensor_tensor(
                x2_sb[:, lt, dc * 512:(dc + 1) * 512], pso,
                xall[:, lt, dc * 512:(dc + 1) * 512], ADD,
            )

    def emit_rms2(lt):
        h2_lt = pde.tile([P, D], bf16, tag="h2bf", bufs=4, name=f"h2bf{lt}")
        _rmsnorm_lt(nc, pde, x2_sb[:, lt, :], h2_lt, eps_sb, "n2", lt)
        return h2_lt

    h2l = []
    for lt in range(LT):
        emit_oproj(lt)
        h2l.append(emit_rms2(lt))

    # dt-major transposes with FFN ft0's gate accumulation interleaved:
    # each dt-matmul only needs h2T's dt column, so the FFN ramp starts as
    # soon as the first four transposes land instead of after all 32
    psg0 = psum.tile([P, L], f32, tag="slotB", bufs=2, name="psg0")
    for dt in range(DT):
        for lt in range(LT):
            tp = psum.tile([P, P], bf16, tag="slotA", bufs=2, name=f"ftp{lt}_{dt}")
            nc.tensor.transpose(tp, h2l[lt][:, dt * P:(dt + 1) * P], ident)
            nc.vector.tensor_copy(h2T[:, dt, lt * P:(lt + 1) * P], tp)
        nc.tensor.matmul(psg0, wg_pre[:, dt, 0:P], h2T[:, dt, :],
                         start=(dt == 0), stop=(dt == DT - 1))

    pde.release()
    pa.release()
    pq.release()
    pexp.release()
    persist1.release()

    # ================= FFN =================
    pfg = tc.alloc_tile_pool(name="ph_fg", bufs=1)
    fT = pfg.tile([P, FTL, L], bf16)          # 32KB/part

    for ft in range(FTL):
        if ft < NPRE:
            wg_c = wg_pre[:, :, ft * P:(ft + 1) * P]
            wu_c = wu_pre[:, :, ft * P:(ft + 1) * P]
        else:
            wg_c = pfg.tile([P, DT, P], bf16, tag="wg", bufs=5, name=f"wg{ft}")
            nc.sync.dma_start(wg_c, wgT_r[:, :, ft * P:(ft + 1) * P])
            wu_c = pfg.tile([P, DT, P], bf16, tag="wu", bufs=5, name=f"wu{ft}")
            nc.sync.dma_start(wu_c, wuT_r[:, :, ft * P:(ft + 1) * P])

        if ft == 0:
            psg = psg0   # accumulated during the transpose interleave above
        else:
            psg = psum.tile([P, L], f32, tag="slotB", bufs=2, name=f"psg{ft}")
            for dt in range(DT):
                nc.tensor.matmul(psg, wg_c[:, dt, :], h2T[:, dt, :],
                                 start=(dt == 0), stop=(dt == DT - 1))
        psu = psum.tile([P, L], f32, tag="slotB", bufs=2, name=f"psu{ft}")
        for dt in range(DT):
            nc.tensor.matmul(psu, wu_c[:, dt, :], h2T[:, dt, :],
                             start=(dt == 0), stop=(dt == DT - 1))
        sl = pfg.tile([P, L], f32, tag="sl", bufs=2, name=f"sl{ft}")
        nc.scalar.activation(sl, psg, mybir.ActivationFunctionType.Sigmoid)
        sl2 = pfg.tile([P, L], f32, tag="sl2", bufs=2, name=f"sl2_{ft}")
        nc.vector.tensor_mul(sl2, sl, psg)
        nc.vector.tensor_mul(fT[:, ft, :], sl2, psu)
        if ft == 0:
            # big down-proj weight DMA on the ACT HWDGE ring: a separate
            # FIFO from the g/u chunk stream, so neither starves the other
            wd_sb = pfg.tile([P, FTL, D], bf16)   # 64KB/part
            nc.scalar.dma_start(wd_sb,
                                wdT_d.ap().rearrange("(ft p) d -> p ft d", p=P))

    # down proj + residual + store
    out_r = out_d.ap().rearrange("(lt p) d -> p lt d", p=P)
    for lt in range(LT):
        o_lt = pfg.tile([P, D], f32, tag="out", bufs=2, name=f"out{lt}")
        for dc in range(D // 512):
            psdn = psum.tile([P, 512], f32, tag="slotC", bufs=2,
                             name=f"psdn{lt}_{dc}")
            for ft in range(FTL):
                nc.tensor.matmul(
                    psdn, fT[:, ft, lt * P:(lt + 1) * P],
                    wd_sb[:, ft, dc * 512:(dc + 1) * 512],
                    start=(ft == 0), stop=(ft == FTL - 1),
                )
            nc.vector.tensor_tensor(
                o_lt[:, dc * 512:(dc + 1) * 512], psdn,
                x2_sb[:, lt, dc * 512:(dc + 1) * 512], ADD,
            )
            # store per-dc so the final 512KB store hides under the last
            # down-proj matmul group instead of trailing the kernel
            nc.sync.dma_start(out_r[:, lt, dc * 512:(dc + 1) * 512],
                              o_lt[:, dc * 512:(dc + 1) * 512])
    pfg.release()
    psum.release()
    persistH.release()
    persist2.release()
    consts.release()


def _to_f8(a):
    return np.ascontiguousarray(np.asarray(a, np.float32).astype(
        ml_dtypes.float8_e4m3))


def _to_bf16(a):
    return np.ascontiguousarray(np.asarray(a, np.float32).astype(
        ml_dtypes.bfloat16))


def prepare_core_inputs(x, text_k, text_v, ln1_w, ln2_w, Wq, Wk, Wv, Wo, Wg, Wu, Wd):
    """Host-side preprocessing: transpose weights, fold RMSNorm gammas, cast."""
    shared = {
        "wqT": _to_f8((np.asarray(Wq) * np.asarray(ln1_w)[None, :]).T),
        "wkT": _to_f8(np.asarray(Wk).T),
        "wvT": _to_f8(np.asarray(Wv).T),
        "woT": _to_f8(np.asarray(Wo).T),
        "wgT": _to_bf16((np.asarray(Wg) * np.asarray(ln2_w)[None, :]).T),
        "wuT": _to_bf16((np.asarray(Wu) * np.asarray(ln2_w)[None, :]).T),
        "wdT": _to_bf16(np.asarray(Wd).T),
    }
    in_maps = []
    for b in range(B):
        in_maps.append({
            "x": _to_bf16(np.asarray(x[b])),
            "tkT": _to_f8(np.asarray(text_k[b]).T),
            "tvT": _to_f8(np.asarray(text_v[b]).T),
            **shared,
        })
    return in_maps


_NC_CACHE = {}


def kernel(**inputs):
    if "nc" not in _NC_CACHE:
        _NC_CACHE["nc"] = build_program()
    nc = _NC_CACHE["nc"]
    in_maps = prepare_core_inputs(**inputs)
    res = run_bass_kernel_spmd(nc, in_maps, core_ids=list(range(B)))
    return np.stack([r["out"] for r in res.results], axis=0)


if __name__ == "__main__":
    # smoke build
    nc = build_program()
    print("program built ok")


# revision 56
# speedup vs baseline: 1.0113x; 1.0113x over previous
"""Trainium2 Bass kernel for ActionExpertCrossBlock (dense transformer block
with GQA cross-attention + SwiGLU FFN), data-parallel over batch on 8 cores.

Contract: kernel(**inputs) takes the FULL fp32 inputs as produced by
setup_inputs() and returns the FULL [8, 512, 1024] fp32 output.

Per-core computation (batch element b):
  h   = rmsnorm(x) * ln1_w
  q   = rope((h @ Wq.T).reshape(L, 8, 256))
  k   = text_k @ Wk.T          (single KV head, shared by all 8 Q heads)
  v   = text_v @ Wv.T
  s_h = q_h @ k.T / 16         -> softmax over context
  ctx = attn @ v ; x2 = ctx @ Wo.T + x
  h2  = rmsnorm(x2) * ln2_w
  out = (silu(h2@Wg.T) * (h2@Wu.T)) @ Wd.T + x2

Precision: the whole attention path (q/k/v projections, scores, exp,
ctx, o-proj) runs in fp8-e4m3 with DoubleRow double-pumped matmuls
(2x PE throughput, K=256 per pass).  The unnormalized softmax runs as
exp(s/16 - 4) so the fp8 range [2^-10, 240] is never exceeded (score
max is ~8 over the full batch); the constant shift cancels in the
softmax ratio because the denominator sums the SAME fp8 exp values.
The FFN (gate/up/down) stays bf16: its outputs are O(1) against a
residual stream whose global absmax is ~6.4, so fp8 there would blow
the 2e-2 absmax gate (measured 0.02-0.04 per matmul in simulation),
while the attention output is tiny relative to the residual and
tolerates fp8 (simulated total rel-err 0.013 vs 0.0025 all-bf16).
x is carried in bf16 (residual quantization ~2e-3 of the gate scale).

Schedule: the attention phase is jointly ACT/PE-bound (after the exp
pair-batching below), the FFN phase PE-bound.  All moving dims stay at
the full 512 columns -- splitting them raises the fixed per-instruction
overheads (ACT ~250ns setup per op, LDWEIGHTS reloads) by more than any
attention/FFN overlap it buys (measured +50us for an L-split variant).
Both score blocks of a context pair accumulate into one 2-bank PSUM
tile so a single 1024-wide ACTIVATE computes their exp (16us less ACT).
Inputs stream in the order wk/wv/tk/x/tv/wq (each dma_start costs
~1.1us of Sync-queue issue time, so the startup loads are chunked
coarsely); q-projection + RoPE for head h+2 is software-pipelined INTO
head h's attention iteration (PSUM slotC + a fast DVE copy to SBUF so
RoPE's DVE work never holds a PSUM bank); the o-proj/gate/up weights
prefetch during the DMA-idle attention phase; v-proj PSUM copies ride
on DVE behind RoPE h0/h1 to keep ACT clear for the exp stream.  Scores
are computed TRANSPOSED ([c, l]) so attn@v needs no transpose of the
attention matrix; the softmax partition-sum is an all-ones fp8
DoubleRow matmul which also broadcasts the denominator to every
partition.
"""
import sys

sys.path.insert(0, "/opt/trn_rl_repo")

import numpy as np
import ml_dtypes

import concourse.bass as bass
from concourse import bacc
import concourse.mybir as mybir
import concourse.tile as tile
from concourse.masks import make_identity
from concourse.bass_utils import run_bass_kernel_spmd

import os as _os

P = 128
B, L, D = 8, 512, 1024
QH, HD = 8, 256
E = 256        # kv dim (1 head x 256)
LC = 2048      # context length
F = 4096       # ffn dim
O = QH * HD    # 2048
LT, DT, OT, CT, FTL = L // P, D // P, O // P, LC // P, F // P  # 4 8 16 16 32
CP = CT // 2   # context pairs for DoubleRow
f32, bf16 = mybir.dt.float32, mybir.dt.bfloat16
f8 = mybir.dt.float8e4
EPS = float(np.finfo(np.float32).eps)
EBIAS = -4.0   # exp(s/16 + EBIAS): keeps fp8 exp in (2^-10, 240)
NPRE = 10      # gate/up weight columns prefetched during attention
EXPF = mybir.ActivationFunctionType.Exp
SQRT = mybir.ActivationFunctionType.Sqrt
COPYF = mybir.ActivationFunctionType.Copy
DR = mybir.MatmulPerfMode.DoubleRow
MUL = mybir.AluOpType.mult
SUB = mybir.AluOpType.subtract
ADD = mybir.AluOpType.add


def _rope_tables():
    # Match reference _rope numerics (fp32 ops) for d=256, l=512.
    d2 = HD // 2
    ts = (10000.0 ** (2.0 / HD * np.arange(d2, dtype=np.float32))).astype(np.float32)
    rad = (np.arange(L, dtype=np.float32)[None, :] / ts[:, None]).astype(np.float32)
    return np.cos(rad).astype(np.float32), np.sin(rad).astype(np.float32)  # [128, 512]


def build_program():
    nc = bacc.Bacc()
    x_d = nc.dram_tensor("x", [L, D], bf16, kind="ExternalInput")
    tkT_d = nc.dram_tensor("tkT", [E, LC], f8, kind="ExternalInput")
    tvT_d = nc.dram_tensor("tvT", [E, LC], f8, kind="ExternalInput")
    wqT_d = nc.dram_tensor("wqT", [D, O], f8, kind="ExternalInput")
    wkT_d = nc.dram_tensor("wkT", [E, E], f8, kind="ExternalInput")
    wvT_d = nc.dram_tensor("wvT", [E, E], f8, kind="ExternalInput")
    woT_d = nc.dram_tensor("woT", [O, D], f8, kind="ExternalInput")
    wgT_d = nc.dram_tensor("wgT", [D, F], bf16, kind="ExternalInput")
    wuT_d = nc.dram_tensor("wuT", [D, F], bf16, kind="ExternalInput")
    wdT_d = nc.dram_tensor("wdT", [F, D], bf16, kind="ExternalInput")
    out_d = nc.dram_tensor("out", [L, D], f32, kind="ExternalOutput")

    ver = np.array([int(_os.environ.get("ANT_KVER", "6"))], np.int32)
    nc.inline_tensor(ver, f"kver_{int(ver[0])}")
    cos_np, sin_np = _rope_tables()
    cosT_d = nc.inline_tensor(cos_np.astype(ml_dtypes.bfloat16), "cosT")
    sinT_d = nc.inline_tensor(sin_np.astype(ml_dtypes.bfloat16), "sinT")

    with tile.TileContext(nc) as tc:
        build_tile_kernel(
            tc, x_d, tkT_d, tvT_d, wqT_d, wkT_d, wvT_d, woT_d, wgT_d, wuT_d,
            wdT_d, cosT_d, sinT_d, out_d,
        )
    nc.compile()
    return nc


def _rmsnorm_lt(nc, pool, x_lt, dst_lt, eps_sb, tag, lt):
    """dst_lt = x_lt * rsqrt(mean(x_lt^2) + eps); sum-of-squares on ACT."""
    sq = pool.tile([P, D], f32, tag=f"{tag}_sq", bufs=2, name=f"{tag}sq{lt}")
    ssum = pool.tile([P, 1], f32, tag=f"{tag}_sum", bufs=2, name=f"{tag}sm{lt}")
    nc.scalar.activation(sq, x_lt, mybir.ActivationFunctionType.Square,
                         accum_out=ssum)
    std = pool.tile([P, 1], f32, tag=f"{tag}_std", bufs=2, name=f"{tag}sd{lt}")
    nc.scalar.activation(std, ssum, SQRT, scale=1.0 / D, bias=eps_sb)
    rstd = pool.tile([P, 1], f32, tag=f"{tag}_rstd", bufs=2, name=f"{tag}rs{lt}")
    nc.vector.reciprocal(rstd, std)
    nc.vector.tensor_scalar_mul(dst_lt, x_lt, rstd)


def build_tile_kernel(tc, x_d, tkT_d, tvT_d, wqT_d, wkT_d, wvT_d, woT_d,
                      wgT_d, wuT_d, wdT_d, cosT_d, sinT_d, out_d):
    nc = tc.nc

    consts = tc.alloc_tile_pool(name="consts", bufs=1)
    persist2 = tc.alloc_tile_pool(name="persist2", bufs=1)  # x2: lives to the end
    persistH = tc.alloc_tile_pool(name="persistH", bufs=1)  # h2T + wg/wu prefetch
    persist1 = tc.alloc_tile_pool(name="persist1", bufs=1)  # dead after o-proj

    ident = consts.tile([P, P], bf16)
    make_identity(nc, ident)
    ones8 = consts.tile([P, 2, P], f8)
    nc.vector.memset(ones8, 1.0)
    eps_sb = consts.tile([P, 1], f32)
    nc.vector.memset(eps_sb, EPS)
    ebias_sb = consts.tile([P, 1], f32)
    nc.vector.memset(ebias_sb, EBIAS)
    cosT = consts.tile([P, L], bf16)
    sinT = consts.tile([P, L], bf16)

    # ---- persistent activations (split into per-slice tiles so consumers
    # depend only on the slices they read, not on whole-tensor last-writes) ----
    qT_h = [persist1.tile([P, 2, L], f8, tag="qT", bufs=QH, name=f"qT{h}")
            for h in range(QH)]
    kT = persist1.tile([P, 2, LC], f8)
    v8 = persist1.tile([P, CP, 2, E], f8)
    ctxT_h = [persist1.tile([P, 2, L], f8, tag="ctxT", bufs=QH, name=f"ctxT{h}")
              for h in range(QH)]
    xall = persist1.tile([P, LT, D], bf16)
    hT = persist1.tile([P, DT, L], f8)
    x2_sb = persist2.tile([P, LT, D], f32)

    pexp = tc.alloc_tile_pool(name="pexp", bufs=1)   # attention exp/recip tiles
    pq = tc.alloc_tile_pool(name="ph_q", bufs=1)     # wq + rope temps
    pa = tc.alloc_tile_pool(name="ph_a", bufs=1)     # startup-only kv inputs
    psum = tc.alloc_tile_pool(name="psum", bufs=2, space="PSUM")

    # ---- input DMAs: each dma_start costs ~1.1us of queue ISSUE time, so
    # the startup-critical loads are spread over three parallel HWDGE rings
    # (Sync: k-side, ACT: x + q weights, DVE: v-side) and chunked coarsely ----
    tk_r = tkT_d.ap().rearrange("(ft p) c -> p ft c", p=P)
    tv_r = tvT_d.ap().rearrange("(ft p) c -> p ft c", p=P)
    x_r = x_d.ap().rearrange("(lt p) d -> p lt d", p=P)
    wq_r = wqT_d.ap().rearrange("(dt p) o -> p dt o", p=P)
    # x first: it feeds the longest pre-attention chain (norm1 -> transpose
    # -> q-proj -> rope) and each issue slot ahead of it costs ~1.1us
    for half in range(2):
        nc.sync.dma_start(xall[:, 2 * half:2 * half + 2, :],
                          x_r[:, 2 * half:2 * half + 2, :])
    nc.sync.dma_start(cosT, cosT_d.ap())
    nc.sync.dma_start(sinT, sinT_d.ap())
    wk_sb = pa.tile([P, 2, E], f8)
    nc.sync.dma_start(wk_sb, wkT_d.ap().rearrange("(ft p) e -> p ft e", p=P))
    wv_sb = pa.tile([P, 2, E], f8)
    nc.sync.dma_start(wv_sb, wvT_d.ap().rearrange("(ft p) e -> p ft e", p=P))
    wq_c = [pq.tile([P, DT, 2 * P], f8, tag="wq", bufs=QH, name=f"wq{h}")
            for h in range(QH)]
    for h in range(2):
        nc.sync.dma_start(wq_c[h], wq_r[:, :, h * 2 * P:(h + 1) * 2 * P])
    tk_c = []
    for cc in range(2):
        t = pa.tile([P, 2, 1024], f8, tag="tk", bufs=2, name=f"tk{cc}")
        nc.sync.dma_start(t, tk_r[:, :, cc * 1024:(cc + 1) * 1024])
        tk_c.append(t)
    tv_c = []
    for cc in range(2):
        t = pa.tile([P, 2, 1024], f8, tag="tv", bufs=2, name=f"tv{cc}")
        nc.sync.dma_start(t, tv_r[:, :, cc * 1024:(cc + 1) * 1024])
        tv_c.append(t)
    for h in range(2, QH):
        nc.sync.dma_start(wq_c[h], wq_r[:, :, h * 2 * P:(h + 1) * 2 * P])

    # ====== rmsnorm1 + transpose first (x-paced) ====
    for lt in range(LT):
        h_lt = pq.tile([P, D], bf16, tag="h_bf", bufs=2, name=f"hbf{lt}")
        _rmsnorm_lt(nc, pq, xall[:, lt, :], h_lt, eps_sb, "n1", lt)
        for dt in range(DT):
            tp = psum.tile([P, P], bf16, tag="slotA", bufs=2, name=f"atp{lt}_{dt}")
            nc.tensor.transpose(tp, h_lt[:, dt * P:(dt + 1) * P], ident)
            nc.vector.tensor_copy(hT[:, dt, lt * P:(lt + 1) * P], tp)

    # ====== k projection (tk-paced, overlaps the norm/transpose chain) ====
    for cc in range(LC // 512):
        for et in range(2):
            psk = psum.tile([P, 512], f32, tag="slotC", bufs=2, name=f"psk{et}_{cc}")
            nc.tensor.matmul(psk, wk_sb[:, :, et * P:(et + 1) * P],
                             tk_c[cc // 2][:, :, (cc % 2) * 512:(cc % 2 + 1) * 512],
                             start=True, stop=True, perf_mode=DR)
            nc.scalar.activation(kT[:, et, cc * 512:(cc + 1) * 512], psk, COPYF)

    # ====== q projection (-> qT, [o, l] layout) + RoPE, per head.
    # psq shares PSUM slotC with the softmax denominator; a fast DVE copy
    # moves each half to SBUF so RoPE's DVE work never holds the PSUM bank. ==
    def emit_qproj(h):
        qsb = []
        for half in range(2):
            psq = psum.tile([P, L], f32, tag="slotC", bufs=2, name=f"psq{2*h+half}")
            for dp in range(DT // 2):
                nc.tensor.matmul(
                    psq, wq_c[h][:, 2 * dp:2 * dp + 2, half * P:(half + 1) * P],
                    hT[:, 2 * dp:2 * dp + 2, :],
                    start=(dp == 0), stop=(dp == DT // 2 - 1), perf_mode=DR)
            qs = pq.tile([P, L], bf16, tag="qsb", bufs=4, name=f"qs{2*h+half}")
            nc.vector.tensor_copy(qs, psq)
            qsb.append(qs)
        # rope: x1 = qsb[0], x2 = qsb[1] ([hd_j, l] layout; tables [j, l]).
        # All-2-byte so DVE runs its 2x mode (bf16 rounding is negligible
        # against the fp8 quantization of qT right after).
        t_a = pq.tile([P, L], bf16, tag="rope_t", bufs=4, name=f"ta{h}")
        nc.vector.tensor_mul(t_a, qsb[0], cosT)
        t_b = pq.tile([P, L], bf16, tag="rope_t", bufs=4, name=f"tb{h}")
        nc.vector.tensor_mul(t_b, qsb[1], sinT)
        nc.vector.tensor_tensor(qT_h[h][:, 0, :], t_a, t_b, SUB)
        t_c = pq.tile([P, L], bf16, tag="rope_t", bufs=4, name=f"tc{h}")
        nc.vector.tensor_mul(t_c, qsb[1], cosT)
        t_d = pq.tile([P, L], bf16, tag="rope_t", bufs=4, name=f"td{h}")
        nc.vector.tensor_mul(t_d, qsb[0], sinT)
        nc.vector.tensor_tensor(qT_h[h][:, 1, :], t_c, t_d, ADD)

    emit_qproj(0)
    emit_qproj(1)

    # ====== v projection (last pre-attention step: v8 is first read by
    # ctx(h0, cp0), a few us into the attention cruise; its PSUM copies ride
    # DVE behind rope h0/h1, keeping ACT clear for the exp stream) ====
    for ct in range(CT):
        psv = psum.tile([P, E], f32, tag="slotC", bufs=2, name=f"psv{ct}")
        nc.tensor.matmul(
            psv, tv_c[ct // 8][:, :, (ct % 8) * P:(ct % 8 + 1) * P], wv_sb,
            start=True, stop=True, perf_mode=DR)
        nc.vector.tensor_copy(v8[:, ct // 2, ct % 2, :], psv)

    # ---- FFN/o-proj weight prefetch: DMA is idle during attention, so gate
    # the bulk loads on DVE-paced dummy writes that fire right about now ----
    pde = tc.alloc_tile_pool(name="ph_de", bufs=1)
    wo8 = pde.tile([P, QH, 2, D], f8)        # 16KB/part
    wgT_r = wgT_d.ap().rearrange("(dt p) f -> p dt f", p=P)
    wuT_r = wuT_d.ap().rearrange("(dt p) f -> p dt f", p=P)
    wg_pre = persistH.tile([P, DT, NPRE * P], bf16)
    wu_pre = persistH.tile([P, DT, NPRE * P], bf16)
    nc.vector.memset(wo8[0:1, 0:1, 0:1, 0:1], 0.0)
    nc.vector.memset(wg_pre[0:1, 0:1, 0:1], 0.0)
    nc.vector.memset(wu_pre[0:1, 0:1, 0:1], 0.0)
    nc.sync.dma_start(wo8, woT_d.ap().rearrange("(h two p) d -> p h two d",
                                                p=P, two=2))
    nc.sync.dma_start(wg_pre, wgT_r[:, :, :NPRE * P])
    nc.sync.dma_start(wu_pre, wuT_r[:, :, :NPRE * P])

    # ---------------- attention (per Q head) ----------------
    for h in range(QH):
        exps = [None] * CP
        psd = psum.tile([P, L], f32, tag="slotC", bufs=2, name=f"psd{h}")
        psc = [
            psum.tile([P, L], f32, tag="slotB", bufs=2, name=f"psc{h}_{et}")
            for et in range(2)
        ]

        def emit_scores(cp, h=h, exps=exps):
            # both score blocks of a context pair land in one 2-bank PSUM
            # tile (separate accumulation groups, each bank-aligned) so a
            # single 1024-wide ACTIVATE computes their exp: ACT's ~250ns
            # per-op setup amortizes over twice the elements
            pss = psum.tile([P, 2, L], f32, tag="slotA", bufs=2,
                            name=f"pss{h}_{cp}")
            for k in range(2):
                nc.tensor.matmul(pss[:, k, :],
                                 kT[:, :, (2 * cp + k) * P:(2 * cp + k + 1) * P],
                                 qT_h[h], start=True, stop=True, perf_mode=DR)
            exps[cp] = pexp.tile([P, 2, L], f8, tag="exp", bufs=4,
                                 name=f"ex{h}_{cp}")
            nc.scalar.activation(exps[cp], pss, EXPF,
                                 scale=1.0 / 16.0, bias=ebias_sb)

        def emit_ctx(cp, psc=psc, exps=exps):
            for et in range(2):
                nc.tensor.matmul(
                    psc[et], v8[:, cp, :, et * P:(et + 1) * P], exps[cp],
                    start=(cp == 0), stop=(cp == CP - 1), perf_mode=DR)

        def emit_den(cp, psd=psd, exps=exps):
            nc.tensor.matmul(psd, ones8, exps[cp],
                             start=(cp == 0), stop=(cp == CP - 1), perf_mode=DR)

        # software pipeline: ctx/den lag scores by one exp-pair so PE never
        # waits on ACT's exp
        for cp in range(CP):
            emit_scores(cp)
            if cp >= 1:
                emit_ctx(cp - 1)
                emit_den(cp - 1)
        emit_ctx(CP - 1)
        emit_den(CP - 1)

        # recip/muls first (they release psd/psc promptly), then head h+2's
        # q-projection rides at end-of-iteration: its RoPE DVE work hides
        # under the next head's ACT-bound exp stream
        recip = pexp.tile([P, L], f32, tag="recip", bufs=2, name=f"rc{h}")
        nc.vector.reciprocal_approx_fast(out=recip, in_=psd)
        for et in range(2):
            nc.vector.tensor_mul(ctxT_h[h][:, et, :], psc[et], recip)

        if h + 2 < QH:
            emit_qproj(h + 2)

    # ------- o-proj + residual, interleaved with norm2/transpose ---------
    h2T = persistH.tile([P, DT, L], bf16)

    def emit_oproj(lt):
        for dc in range(D // 512):
            pso = psum.tile([P, 512], f32, tag="slotC", bufs=2, name=f"pso{lt}_{dc}")
            for h in range(QH):
                nc.tensor.matmul(
                    pso, ctxT_h[h][:, :, lt * P:(lt + 1) * P],
                    wo8[:, h, :, dc * 512:(dc + 1) * 512],
                    start=(h == 0), stop=(h == QH - 1), perf_mode=DR)
            nc.vector.tensor_tensor(
                x2_sb[:, lt, dc * 512:(dc + 1) * 512], pso,
                xall[:, lt, dc * 512:(dc + 1) * 512], ADD,
            )

    def emit_norm2(lt):
        h2_lt = pde.tile([P, D], bf16, tag="h2bf", bufs=2, name=f"h2bf{lt}")
        _rmsnorm_lt(nc, pde, x2_sb[:, lt, :], h2_lt, eps_sb, "n2", lt)
        for dt in range(DT):
            tp = psum.tile([P, P], bf16, tag="slotA", bufs=2, name=f"ftp{lt}_{dt}")
            nc.tensor.transpose(tp, h2_lt[:, dt * P:(dt + 1) * P], ident)
            nc.vector.tensor_copy(h2T[:, dt, lt * P:(lt + 1) * P], tp)

    emit_oproj(0)
    for lt in range(1, LT):
        emit_oproj(lt)
        emit_norm2(lt - 1)
    emit_norm2(LT - 1)

    pde.release()
    pa.release()
    pq.release()
    pexp.release()
    persist1.release()

    # ================= FFN =================
    pfg = tc.alloc_tile_pool(name="ph_fg", bufs=1)
    fT = pfg.tile([P, FTL, L], bf16)          # 32KB/part

    for ft in range(FTL):
        if ft < NPRE:
            wg_c = wg_pre[:, :, ft * P:(ft + 1) * P]
            wu_c = wu_pre[:, :, ft * P:(ft + 1) * P]
        else:
            wg_c = pfg.tile([P, DT, P], bf16, tag="wg", bufs=5, name=f"wg{ft}")
            nc.sync.dma_start(wg_c, wgT_r[:, :, ft * P:(ft + 1) * P])
            wu_c = pfg.tile([P, DT, P], bf16, tag="wu", bufs=5, name=f"wu{ft}")
            nc.sync.dma_start(wu_c, wuT_r[:, :, ft * P:(ft + 1) * P])

        psg = psum.tile([P, L], f32, tag="slotB", bufs=2, name=f"psg{ft}")
        for dt in range(DT):
            nc.tensor.matmul(psg, wg_c[:, dt, :], h2T[:, dt, :],
                             start=(dt == 0), stop=(dt == DT - 1))
        psu = psum.tile([P, L], f32, tag="slotB", bufs=2, name=f"psu{ft}")
        for dt in range(DT):
            nc.tensor.matmul(psu, wu_c[:, dt, :], h2T[:, dt, :],
                             start=(dt == 0), stop=(dt == DT - 1))
        sl = pfg.tile([P, L], f32, tag="sl", bufs=2, name=f"sl{ft}")
        nc.scalar.activation(sl, psg, mybir.ActivationFunctionType.Sigmoid)
        sl2 = pfg.tile([P, L], f32, tag="sl2", bufs=2, name=f"sl2_{ft}")
        nc.vector.tensor_mul(sl2, sl, psg)
        nc.vector.tensor_mul(fT[:, ft, :], sl2, psu)
        if ft == 0:
            # big down-proj weight DMA on the ACT HWDGE ring: a separate
            # FIFO from the g/u chunk stream, so neither starves the other
            wd_sb = pfg.tile([P, FTL, D], bf16)   # 64KB/part
            nc.scalar.dma_start(wd_sb,
                                wdT_d.ap().rearrange("(ft p) d -> p ft d", p=P))

    # down proj + residual + store
    out_r = out_d.ap().rearrange("(lt p) d -> p lt d", p=P)
    for lt in range(LT):
        o_lt = pfg.tile([P, D], f32, tag="out", bufs=2, name=f"out{lt}")
        for dc in range(D // 512):
            psdn = psum.tile([P, 512], f32, tag="slotC", bufs=2,
                             name=f"psdn{lt}_{dc}")
            for ft in range(FTL):
                nc.tensor.matmul(
                    psdn, fT[:, ft, lt * P:(lt + 1) * P],
                    wd_sb[:, ft, dc * 512:(dc + 1) * 512],
                    start=(ft == 0), stop=(ft == FTL - 1),
                )
            nc.vector.tensor_tensor(
                o_lt[:, dc * 512:(dc + 1) * 512], psdn,
                x2_sb[:, lt, dc * 512:(dc + 1) * 512], ADD,
            )
            # store per-dc so the final 512KB store hides under the last
            # down-proj matmul group instead of trailing the kernel
            nc.sync.dma_start(out_r[:, lt, dc * 512:(dc + 1) * 512],
                              o_lt[:, dc * 512:(dc + 1) * 512])
    pfg.release()
    psum.release()
    persistH.release()
    persist2.release()
    consts.release()


def _to_f8(a):
    return np.ascontiguousarray(np.asarray(a, np.float32).astype(
        ml_dtypes.float8_e4m3))


def _to_bf16(a):
    return np.ascontiguousarray(np.asarray(a, np.float32).astype(
        ml_dtypes.bfloat16))


def prepare_core_inputs(x, text_k, text_v, ln1_w, ln2_w, Wq, Wk, Wv, Wo, Wg, Wu, Wd):
    """Host-side preprocessing: transpose weights, fold RMSNorm gammas, cast."""
    shared = {
        "wqT": _to_f8((np.asarray(Wq) * np.asarray(ln1_w)[None, :]).T),
        "wkT": _to_f8(np.asarray(Wk).T),
        "wvT": _to_f8(np.asarray(Wv).T),
        "woT": _to_f8(np.asarray(Wo).T),
        "wgT": _to_bf16((np.asarray(Wg) * np.asarray(ln2_w)[None, :]).T),
        "wuT": _to_bf16((np.asarray(Wu) * np.asarray(ln2_w)[None, :]).T),
        "wdT": _to_bf16(np.asarray(Wd).T),
    }
    in_maps = []
    for b in range(B):
        in_maps.append({
            "x": _to_bf16(np.asarray(x[b])),
            "tkT": _to_f8(np.asarray(text_k[b]).T),
            "tvT": _to_f8(np.asarray(text_v[b]).T),
            **shared,
        })
    return in_maps


_NC_CACHE = {}


def kernel(**inputs):
    if "nc" not in _NC_CACHE:
        _NC_CACHE["nc"] = build_program()
    nc = _NC_CACHE["nc"]
    in_maps = prepare_core_inputs(**inputs)
    res = run_bass_kernel_spmd(nc, in_maps, core_ids=list(range(B)))
    return np.stack([r["out"] for r in res.results], axis=0)


if __name__ == "__main__":
    # smoke build
    nc = build_program()
    print("program built ok")
